# revision 1
# baseline (speedup 1.0000x reference)
"""Single-head attention (S=8192, D=1024, d_k=128) on 8 TRN2 NeuronCores.

Strategy: sequence-parallel. Each core owns SL=1024 query rows. Per core:
  - transpose x shard via PE identity matmuls to get x^T tiles
  - project K^T, V^T locally; PE-transpose V^T -> V natural; AllGather K^T
    and V-nat across the 8 cores (DRAM collectives)
  - project Q^T locally
  - per 512-query block:
      phase 1: S = Q^T.T @ K^T in natural [q, k] orientation, free-dim
               reduce_max from PSUM -> row max m (exact)
      phase 2: S^T = K^T-block.T @ Q^T in [k, q] orientation; DVE fuses
               scale + subtract(m) (m broadcast across partitions via a
               K=1 ones matmul), ACT exps into P^T, which feeds the PV
               matmul with no transpose: O^T += V-block.T @ P^T, and
               l += ones.T @ P^T
      normalize by 1/l, PE-transpose O^T -> O, DMA out.

Toolchain constraint: walrus allows at most ONE sync wait per Matmult.
Discipline used here:
  - every DMA-fed matmul operand gets a tiny "absorber" matmul right
    after its DMA (1 wait: that DMA queue sem), folding the DMA tick
    into PE's vector clock so real matmuls never wait on DMA;
  - PSUM slots are pooled so a matmul's WAR wait lands on the same
    engine semaphore as its data input wait (max-merged into one).
"""

import math
import os
import sys
from contextlib import ExitStack

for _p in ("/opt/trn_rl_repo", os.path.expanduser("~/.axon_site/_ro/trn_rl_repo")):
    if os.path.isdir(_p) and _p not in sys.path:
        sys.path.insert(0, _p)

import numpy as np

import concourse.bass as bass
import concourse.mybir as mybir
import concourse.tile as tile
from concourse.bass_utils import run_bass_kernel_spmd
from concourse.masks import make_identity

S = 8192
D = 1024
DK = 128
NC = 8
SL = S // NC  # 1024 query rows per core
SCALE = 1.0 / math.sqrt(DK)
FP32 = mybir.dt.float32
Act = mybir.ActivationFunctionType
Alu = mybir.AluOpType


def build_program() -> bass.Bass:
    nc = bass.Bass(num_devices=NC)

    x_sh = nc.declare_dram_parameter("x_sh", [SL, D], FP32, isOutput=False)
    w_q = nc.declare_dram_parameter("W_Q", [D, DK], FP32, isOutput=False)
    b_q = nc.declare_dram_parameter("b_Q", [1, DK], FP32, isOutput=False)
    w_k = nc.declare_dram_parameter("W_K", [D, DK], FP32, isOutput=False)
    b_k = nc.declare_dram_parameter("b_K", [1, DK], FP32, isOutput=False)
    w_v = nc.declare_dram_parameter("W_V", [D, DK], FP32, isOutput=False)
    b_v = nc.declare_dram_parameter("b_V", [1, DK], FP32, isOutput=False)
    out_sh = nc.declare_dram_parameter("out_sh", [SL, DK], FP32, isOutput=True)

    groups = [list(range(NC))]

    with tile.TileContext(nc) as tc, ExitStack() as ctx:
        dram = ctx.enter_context(tc.tile_pool(name="dram", bufs=1, space="DRAM"))
        ktl_d = dram.tile([DK, SL], FP32)
        ktg_d = dram.tile([NC * DK, SL], FP32, addr_space="Shared")
        vnl_d = dram.tile([SL, DK], FP32)
        vng_d = dram.tile([S, DK], FP32, addr_space="Shared")
        const = ctx.enter_context(tc.tile_pool(name="const", bufs=1))
        big = ctx.enter_context(tc.tile_pool(name="big", bufs=1))
        stat = ctx.enter_context(tc.tile_pool(name="stat", bufs=2))
        work = ctx.enter_context(tc.tile_pool(name="work", bufs=2))
        outp = ctx.enter_context(tc.tile_pool(name="outp", bufs=3))
        # PSUM budget (8 banks): psS 3 + psO 2 + psL 2 + psB 1
        psS = ctx.enter_context(tc.tile_pool(name="psS", bufs=5, space="PSUM"))
        psacc = ctx.enter_context(tc.tile_pool(name="psacc", bufs=2, space="PSUM"))
        psB = ctx.enter_context(tc.tile_pool(name="psB", bufs=1, space="PSUM"))

        def absorb(col_ap):
            """1-wait PE ldweights folding col_ap's producer sem into PE's clock.

            Bare InstLdweights has no output, so it builds no WAW chain; the
            bf16 bitcast sidesteps the fp32 standalone-ldweights restriction
            (the loaded garbage weights are never used -- every real matmul
            self-loads since ldw-opt is disabled).
            """
            nc.tensor.ldweights(weights=col_ap.bitcast(mybir.dt.bfloat16))

        ident = const.tile([128, 128], FP32)
        make_identity(nc, ident[:, :])
        absorb(ident[:, 0:1])
        ones = const.tile([128, 1], FP32)
        nc.gpsimd.memset(ones[:, :], 1.0)
        absorb(ones[:, 0:1])
        ones_row = const.tile([1, 128], FP32, tag="ones_row")
        nc.gpsimd.memset(ones_row[:, :], 1.0)
        absorb(ones_row[0:1, 0:1])

        def load_bias_T(b_dram, tag):
            t = const.tile([128, 1], FP32, tag=tag)
            nc.sync.dma_start(out=t[:, 0], in_=b_dram[0, :])
            return t

        bqT = load_bias_T(b_q, "bqT")
        bkT = load_bias_T(b_k, "bkT")
        bvT = load_bias_T(b_v, "bvT")

        proj_ctx = ExitStack()
        xpool = proj_ctx.enter_context(tc.tile_pool(name="xpool", bufs=1))
        xload = proj_ctx.enter_context(tc.tile_pool(name="xload", bufs=8))
        wpool = proj_ctx.enter_context(tc.tile_pool(name="wpool", bufs=3))
        # x^T: [128 d-part, 8 d-tiles, SL seq]  (d = dt*128 + partition)
        xT = xpool.tile([128, D // 128, SL], FP32)
        for st in range(SL // 128):
            xn = xload.tile([128, D], FP32)
            nc.gpsimd.dma_start(out=xn[:, :], in_=x_sh[st * 128 : (st + 1) * 128, :])
            absorb(xn[:, 0:1])
            for dt in range(D // 128):
                pt = psS.tile([128, 128], FP32, tag="ps")
                nc.tensor.matmul(pt[:, :], lhsT=xn[:, dt * 128 : (dt + 1) * 128], rhs=ident[:, :], start=True, stop=True)
                nc.scalar.copy(xT[:, dt, st * 128 : (st + 1) * 128], pt[:, :])

        def proj(w_dram, bT, outT):
            """outT [128 dk, SL] = (x_local @ W + b)^T"""
            wt = wpool.tile([128, D // 128, DK], FP32)
            nc.gpsimd.dma_start(out=wt[:, :, :], in_=w_dram.rearrange("(t p) k -> p t k", p=128))
            absorb(wt[:, 0, 0:1])
            for g in range(SL // 512):
                ps = psS.tile([128, 512], FP32, tag="ps")
                for dt in range(D // 128):
                    nc.tensor.matmul(
                        ps[:, :],
                        lhsT=wt[:, dt, :],
                        rhs=xT[:, dt, g * 512 : (g + 1) * 512],
                        start=(dt == 0),
                        stop=(dt == D // 128 - 1),
                    )
                nc.scalar.activation(outT[:, g * 512 : (g + 1) * 512], ps[:, :], Act.Identity, bias=bT[:, :])

        # K first so its collective starts early
        ktl = big.tile([128, SL], FP32)
        proj(w_k, bkT, ktl)
        nc.gpsimd.dma_start(out=ktl_d[:, :], in_=ktl[:, :])
        nc.gpsimd.collective_compute(
            "AllGather", Alu.bypass, replica_groups=groups, ins=[ktl_d[:, :]], outs=[ktg_d[:, :]]
        )

        vtl = big.tile([128, SL], FP32)
        proj(w_v, bvT, vtl)
        vnl = big.tile([128, SL // 128, DK], FP32)
        for st in range(SL // 128):
            pt = psS.tile([128, 128], FP32, tag="ps")
            nc.tensor.matmul(pt[:, :], lhsT=vtl[:, st * 128 : (st + 1) * 128], rhs=ident[:, :], start=True, stop=True)
            nc.scalar.copy(vnl[:, st, :], pt[:, :])
        nc.gpsimd.dma_start(out=vnl_d.rearrange("(t p) k -> p t k", p=128), in_=vnl[:, :, :])
        nc.gpsimd.collective_compute(
            "AllGather", Alu.bypass, replica_groups=groups, ins=[vnl_d[:, :]], outs=[vng_d[:, :]]
        )

        qT = big.tile([128, SL], FP32)
        proj(w_q, bqT, qT)
        proj_ctx.close()

        # gathered K^T [128 dk, 8192 ks] and V natural [128 ks-part, 64 tiles, 128 dk]
        ktF = big.tile([128, NC, SL], FP32)
        nc.gpsimd.dma_start(out=ktF[:, :, :], in_=ktg_d.rearrange("(c p) s -> p c s", p=128))
        absorb(ktF[:, 0, 0:1])
        ktF2 = ktF.rearrange("p c s -> p (c s)")
        vnF = big.tile([128, S // 128, DK], FP32)
        nc.gpsimd.dma_start(out=vnF[:, :, :], in_=vng_d.rearrange("(t p) k -> p t k", p=128))
        absorb(vnF[:, 0, 0:1])

        NQ = 256  # queries per block
        sraw_pool = ctx.enter_context(tc.tile_pool(name="sraw", bufs=1))
        sraw = sraw_pool.tile([128, S // 128, NQ], FP32)
        for g in range(SL // NQ):
            qs = slice(g * NQ, (g + 1) * NQ)
            # ---- pass 1: S^T tiles -> scaled raw scores in SBUF + running max ----
            macc = stat.tile([128, NQ], FP32)
            for kt in range(S // 128):
                ps = psS.tile([128, NQ], FP32, tag="ps")
                nc.tensor.matmul(
                    ps[:, :], lhsT=ktF2[:, kt * 128 : (kt + 1) * 128], rhs=qT[:, qs],
                    start=True, stop=True,
                )
                nc.scalar.mul(sraw[:, kt, :], ps[:, :], SCALE)
                if kt == 0:
                    nc.vector.tensor_copy(macc[:, :], sraw[:, 0, :])
                else:
                    nc.vector.tensor_max(macc[:, :], macc[:, :], sraw[:, kt, :])
            # reduce over the 128 lanes: PE-transpose each 128-q chunk, then
            # free-dim reduce_max (negated) and transpose the result to a row
            mrow = stat.tile([1, NQ], FP32)
            for qt in range(NQ // 128):
                ptr = psS.tile([128, 128], FP32, tag="ps")
                nc.tensor.matmul(ptr[:, :], lhsT=macc[:, qt * 128 : (qt + 1) * 128], rhs=ident[:, :], start=True, stop=True)
                mq = stat.tile([128, 1], FP32, tag="mq")
                nc.vector.reduce_max(mq[:, :], ptr[:, :], axis=mybir.AxisListType.X, negate=True)
                prm = psS.tile([128, 128], FP32, tag="ps")
                nc.tensor.matmul(prm[0:1, :], lhsT=mq[:, :], rhs=ident[:, :], start=True, stop=True)
                nc.scalar.copy(mrow[0:1, qt * 128 : (qt + 1) * 128], prm[0:1, :])
            mbc = stat.tile([128, NQ], FP32)
            pb = psB.tile([128, NQ], FP32, tag="bc")
            nc.tensor.matmul(pb[:, :], lhsT=ones_row[0:1, :], rhs=mrow[0:1, :], start=True, stop=True)
            nc.vector.tensor_copy(mbc[:, :], pb[:, :])

            # ---- pass 2: exp, PV accumulate, row sums on gpsimd ----
            # elementwise chains run on 4-kt batches ([128, 4*NQ]) to amortize
            # per-op fixed overhead; PV slices the batched ptile per kt
            psO = psacc.tile([128, NQ], FP32)
            lacc = stat.tile([128, 4, NQ], FP32, tag="lacc", bufs=1)
            KB = 4
            for kb in range(S // 128 // KB):
                ssc = work.tile([128, KB, NQ], FP32, tag="ssc")
                for h in range(KB):
                    nc.vector.tensor_add(ssc[:, h, :], sraw[:, kb * KB + h, :], mbc[:, :])
                ptile = work.tile([128, KB, NQ], FP32, tag="ptile")
                nc.scalar.activation(ptile[:, :, :], ssc[:, :, :], Act.Exp)
                for h in range(KB):
                    kt = kb * KB + h
                    nc.tensor.matmul(
                        psO[:, :], lhsT=vnF[:, kt, :], rhs=ptile[:, h, :],
                        start=(kt == 0), stop=(kt == S // 128 - 1), skip_group_check=True,
                    )
                if kb == 0:
                    nc.gpsimd.tensor_copy(lacc[:, :, :], ptile[:, :, :])
                else:
                    nc.gpsimd.tensor_add(lacc[:, :, :], lacc[:, :, :], ptile[:, :, :])
            for h in range(1, 4):
                nc.vector.tensor_add(lacc[:, 0, :], lacc[:, 0, :], lacc[:, h, :])
            rrow = stat.tile([1, NQ], FP32)
            for qt in range(NQ // 128):
                ptr = psS.tile([128, 128], FP32, tag="ps")
                nc.tensor.matmul(ptr[:, :], lhsT=lacc[:, 0, qt * 128 : (qt + 1) * 128], rhs=ident[:, :], start=True, stop=True)
                lq = stat.tile([128, 1], FP32, tag="lq")
                nc.vector.reduce_sum(lq[:, :], ptr[:, :], axis=mybir.AxisListType.X)
                rq = stat.tile([128, 1], FP32, tag="rq")
                nc.vector.reciprocal(rq[:, :], lq[:, :])
                prm = psS.tile([128, 128], FP32, tag="ps")
                nc.tensor.matmul(prm[0:1, :], lhsT=rq[:, :], rhs=ident[:, :], start=True, stop=True)
                nc.scalar.copy(rrow[0:1, qt * 128 : (qt + 1) * 128], prm[0:1, :])
            rbc = stat.tile([128, NQ], FP32)
            pb2 = psB.tile([128, NQ], FP32, tag="bc")
            nc.tensor.matmul(pb2[:, :], lhsT=ones_row[0:1, :], rhs=rrow[0:1, :], start=True, stop=True)
            nc.vector.tensor_copy(rbc[:, :], pb2[:, :])
            otn = work.tile([128, NQ], FP32, tag="otn")
            nc.vector.tensor_mul(otn[:, :], psO[:, :], rbc[:, :])
            for qt in range(NQ // 128):
                po = psS.tile([128, 128], FP32, tag="ps")
                nc.tensor.matmul(po[:, :], lhsT=otn[:, qt * 128 : (qt + 1) * 128], rhs=ident[:, :], start=True, stop=True)
                ot = outp.tile([128, 128], FP32)
                nc.scalar.copy(ot[:, :], po[:, :])
                q0 = g * NQ + qt * 128
                nc.gpsimd.dma_start(out=out_sh[q0 : q0 + 128, :], in_=ot[:, :])

    split_multi_waits(nc)
    return nc


def split_multi_waits(nc):
    """Hoist all-but-one sync wait off engine/DMA instructions into
    standalone EventSemaphore instructions.

    This toolchain's walrus build has a single wait-command slot per
    non-sequencer instruction; Tile emits multi-wait instructions assuming
    a newer codegen. Sequencer sync instructions (EventSemaphore, Drain)
    accept arbitrary waits, so semantics are preserved by hoisting.
    """
    import bass_rust

    exempt = {"InstEventSemaphore"}
    n_split = 0
    for f in nc.m.functions:
        for bb in f.blocks:
            out = []
            changed = False
            for ins in bb.instructions:
                si = ins.sync_info
                if (
                    si is not None
                    and len(si.on_wait) > 1
                    and type(ins).__name__ not in exempt
                    and ins.engine is not None
                ):
                    for j, w in enumerate(si.on_wait[:-1]):
                        ev = mybir.InstEventSemaphore(
                            name=f"{ins.name}-wsplit{j}", ins=[], outs=[]
                        )
                        ev.engine = ins.engine
                        ev.sync_info = bass_rust.SyncInfo(on_wait=[w], on_update=[])
                        out.append(ev)
                        n_split += 1
                    ins.sync_info = bass_rust.SyncInfo(
                        on_wait=[si.on_wait[-1]], on_update=list(si.on_update)
                    )
                    changed = True
                out.append(ins)
            if changed:
                bb.instructions = out
    return n_split


_PROGRAM = None


def _get_program():
    global _PROGRAM
    if _PROGRAM is None:
        _PROGRAM = build_program()
    return _PROGRAM


def kernel(x, W_Q, b_Q, W_K, b_K, W_V, b_V):
    x = np.ascontiguousarray(np.asarray(x, dtype=np.float32))
    args = {
        "W_Q": np.ascontiguousarray(np.asarray(W_Q, dtype=np.float32)),
        "b_Q": np.ascontiguousarray(np.asarray(b_Q, dtype=np.float32)),
        "W_K": np.ascontiguousarray(np.asarray(W_K, dtype=np.float32)),
        "b_K": np.ascontiguousarray(np.asarray(b_K, dtype=np.float32)),
        "W_V": np.ascontiguousarray(np.asarray(W_V, dtype=np.float32)),
        "b_V": np.ascontiguousarray(np.asarray(b_V, dtype=np.float32)),
    }
    nc = _get_program()
    in_maps = [dict(args, x_sh=x[c * SL : (c + 1) * SL]) for c in range(NC)]
    res = run_bass_kernel_spmd(nc, in_maps, list(range(NC)))
    return np.concatenate([res.results[c]["out_sh"] for c in range(NC)], axis=0)

